# revision 14
# baseline (speedup 1.0000x reference)
"""Trainium2 Bass kernel for the BGNN (3-layer GCN x 2 branches + mean-pool + MLP).

v2 design (ap_gather-based):
  - Nodes dst-sharded across 8 cores (SH=12544/core). Per branch, edges split
    into 8 gather streams by src%8; each GpSimd Q7 core gathers its stream's
    source features from an SBUF-resident replicated table via ap_gather
    (features transposed onto partitions, 8-node column packing).
  - L1 table bf16 feat-pairs in u32 units ([128, NT, 2] bf16); L2 table f32
    [128, NT]. Tables device-built per band, AllGathered, reloaded packed.
  - Per 128-edge-slot block: TensorE strided-plane transposes flip [feat,edge]
    to [edge,feat]; dl-vs-iota one-hot matmuls scatter into per-window PSUM
    accumulators (bank-first start, bank-last stop).
  - Layer 3 + mean-pool folded into host-precomputed structural matrix
    Dt[s,g]: pool = ((Dt^T @ act2)/cnt) @ W3 + b3.  MLP replicated per core.
  - Block->window schedule baked into the SPMD program: per-(stream,window)
    run capacities common across cores (max over cores, padded to 32);
    window-straddling blocks use multiple masked one-hot fragments.
"""
import sys

sys.path.insert(0, "/opt/trn_rl_repo")

import numpy as np
import ml_dtypes

import concourse.bacc as bacc
import concourse.bass as bass
import concourse.mybir as mybir
import concourse.tile as tile
from concourse.bass_utils import run_bass_kernel_spmd

P = 128
NCORE = 8
G = 64
N = 100000
SH = 12544
NPAD = SH * NCORE
NW = SH // P                # 98
NT = NPAD // 8              # 12544
CH = SH // 8                # 1568
KI = 4096
PI = np.concatenate([np.arange(0, 32, 2), np.arange(1, 32, 2)])

bfloat16 = ml_dtypes.bfloat16
last_results = None


def _pad_to(x, m):
    return (x + m - 1) // m * m


# --------------------------------------------------------------------------
# host prep
# --------------------------------------------------------------------------

def _prep_branch(ei, batch):
    src = ei[0].astype(np.int64)
    dst = ei[1].astype(np.int64)
    deg = np.bincount(dst, minlength=N).astype(np.float32) + 1.0
    dis = np.ones(NPAD, np.float32)
    dis[:N] = deg ** -0.5

    bpad = np.zeros(NPAD, np.int64)
    bpad[:N] = batch.astype(np.int64)
    flat = np.bincount(src * G + bpad[dst], weights=dis[dst].astype(np.float64),
                      minlength=NPAD * G)
    Dt = flat.reshape(NPAD, G).astype(np.float32)
    Dt *= dis[:, None]
    Dt[np.arange(N), bpad[:N]] += dis[:N] ** 2
    cnt = np.bincount(batch.astype(np.int64), minlength=G).astype(np.float32)
    inv_cnt = (1.0 / np.maximum(cnt, 1.0)).reshape(G, 1)

    core = dst // SH
    loc_s = src % SH
    stream = loc_s % 8
    gidx = (src // SH) * CH + loc_s // 8
    win = (dst % SH) // P
    dlv = (dst % P).astype(np.float32)

    key = (core * 8 + stream) * NW + win
    counts = np.bincount(key, minlength=NCORE * 8 * NW).reshape(NCORE, 8, NW)
    caps = _pad_to(counts.max(axis=0), 4)
    caps[0] = np.maximum(caps[0], 4)
    L = int(_pad_to(caps.sum(axis=1).max(), KI // 8 if False else P))
    NB = L // P

    starts = np.zeros((8, NW), np.int64)
    for q in range(8):
        np.cumsum(caps[q][:-1], out=starts[q][1:])

    # common block schedule
    frag_win = [[] for _ in range(NB * 8)]
    for q in range(8):
        for w in range(NW):
            s0, s1 = starts[q][w], starts[q][w] + caps[q][w]
            for k in range(s0 // P, min((s1 + P - 1) // P, NB)):
                frag_win[k * 8 + q].append(w)
    dlcols = []
    sched = []
    for t in range(NB * 8):
        kk, q = t // 8, t % 8
        ent = []
        for w in frag_win[t]:
            col = len(dlcols)
            s0, s1 = starts[q][w], starts[q][w] + caps[q][w]
            lo, hi = max(s0, kk * P), min(s1, (kk + 1) * P)
            dlcols.append((t, q, w, lo, hi))
            ent.append((col, w))
        sched.append(ent)
    TB = _pad_to(len(dlcols), 32)

    order = np.lexsort((win, stream, core))
    gidx_o, dl_o, win_o = gidx[order], dlv[order], win[order]
    core_o, stream_o = core[order], stream[order]
    cbounds = np.searchsorted(core_o, np.arange(NCORE + 1))
    per_core = []
    for c in range(NCORE):
        lo, hi = cbounds[c], cbounds[c + 1]
        gq, gw = stream_o[lo:hi], win_o[lo:hi]
        gi, gd = gidx_o[lo:hi], dl_o[lo:hi]
        idx_arr = np.zeros((8, L), np.int64)
        dl_full = np.full((8, NB * P), -1.0, np.float32)
        qb = np.searchsorted(gq, np.arange(9))
        for q in range(8):
            ql, qh = qb[q], qb[q + 1]
            wq, iq, dq = gw[ql:qh], gi[ql:qh], gd[ql:qh]
            wcnt = np.bincount(wq, minlength=NW)
            wstart = np.zeros(NW, np.int64)
            np.cumsum(wcnt[:-1], out=wstart[1:])
            pos = starts[q][wq] + (np.arange(qh - ql) - wstart[wq])
            idx_arr[q, pos] = iq
            dl_full[q, pos] = dq
        idx_tile = np.zeros((P, L // 16), np.int16)
        for g in range(8):
            idx_tile[16 * g:16 * g + 16, :] = \
                idx_arr[g].astype(np.int16).reshape(L // 16, 16).T
        dl_tile = np.full((P, TB), -1.0, np.float32)
        for col, (t, q, w, flo, fhi) in enumerate(dlcols):
            kk = t // 8
            seg = dl_full[q, kk * P:(kk + 1) * P].copy()
            mask = np.zeros(P, bool)
            mask[flo - kk * P:fhi - kk * P] = True
            seg[~mask] = -1.0
            dl_tile[:, col] = seg
        dis_t = np.ascontiguousarray(dis[c * SH:(c + 1) * SH].reshape(NW, P).T)
        disP = np.ascontiguousarray(dis[c * SH:(c + 1) * SH].reshape(CH, 8).T)
        DtT = np.ascontiguousarray(
            Dt[c * SH:(c + 1) * SH].reshape(NW, P, G).transpose(1, 0, 2)
            .reshape(P, NW * G)).astype(bfloat16)
        per_core.append(dict(idx=idx_tile, dl=dl_tile.astype(bfloat16),
                             dist=dis_t, dist2=dis_t * dis_t, disP=disP,
                             DtT=DtT))
    return dict(sched=sched, TB=TB, NB=NB, L=L, inv_cnt=inv_cnt,
                per_core=per_core, dis=dis)


# --------------------------------------------------------------------------
# device program
# --------------------------------------------------------------------------

def _build_program(schs):
    nc = bacc.Bacc()
    f32 = mybir.dt.float32
    bf16 = mybir.dt.bfloat16
    i16 = mybir.dt.int16

    prm = {}
    for b in (0, 1):
        sch = schs[b]
        prm[f"xT{b}"] = nc.declare_dram_parameter(f"xT{b}", [P, SH], bf16, isOutput=False)
        prm[f"idx{b}"] = nc.declare_dram_parameter(f"idx{b}", [P, sch["L"] // 16], i16, isOutput=False)
        prm[f"dl{b}"] = nc.declare_dram_parameter(f"dl{b}", [P, sch["TB"]], bf16, isOutput=False)
        prm[f"dist{b}"] = nc.declare_dram_parameter(f"dist{b}", [P, NW], f32, isOutput=False)
        prm[f"dist2{b}"] = nc.declare_dram_parameter(f"dist2{b}", [P, NW], f32, isOutput=False)
        prm[f"DtT{b}"] = nc.declare_dram_parameter(f"DtT{b}", [P, NW * G], bf16, isOutput=False)
        prm[f"ic{b}"] = nc.declare_dram_parameter(f"ic{b}", [G, 1], f32, isOutput=False)
    for nm, shp, dt in (
        ("W1eo", [P, 32], bf16), ("W1pi", [P, 32], bf16), ("b1rep", [P, 32], f32),
        ("W2p", [32, 16], bf16), ("b2rep", [P, 16], f32),
        ("W3", [16, 8], f32), ("b3r", [G, 8], f32),
        ("mW1", [16, 8], f32), ("mb1r", [G, 8], f32),
        ("mW2", [8, 2], f32), ("mb2r", [G, 2], f32),
        ("identf", [P, P], f32), ("iota", [P, P], bf16),
    ):
        prm[nm] = nc.declare_dram_parameter(nm, shp, dt, isOutput=False)
    out_p = nc.declare_dram_parameter("out", [G, 2], f32, isOutput=True)

    t1loc = [nc.dram_tensor(f"t1loc{b}", [P, 2 * CH], bf16) for b in (0, 1)]
    t1full = [nc.dram_tensor(f"t1full{b}", [NCORE * P, 2 * CH], bf16) for b in (0, 1)]
    t2loc = [nc.dram_tensor(f"t2loc{b}", [P, CH], f32) for b in (0, 1)]
    t2full = [nc.dram_tensor(f"t2full{b}", [NCORE * P, CH], f32) for b in (0, 1)]
    pool_in = nc.dram_tensor("pool_in", [G, 32], f32)
    pool_out = nc.dram_tensor("pool_out", [G, 32], f32)

    with tile.TileContext(nc) as tc:
        with (
            tc.tile_pool(name="const", bufs=1) as cp,
            tc.tile_pool(name="tabs", bufs=1) as tbp,
            tc.tile_pool(name="stream", bufs=2) as sp,
            tc.tile_pool(name="small", bufs=3) as sm,
            tc.tile_pool(name="auxp", bufs=1, space="PSUM") as auxp,
        ):
            ct = {}
            for nm in ("W1eo", "W1pi", "b1rep", "W2p", "b2rep", "W3", "b3r",
                       "mW1", "mb1r", "mW2", "mb2r", "identf", "iota"):
                t = cp.tile(list(prm[nm].shape), prm[nm].dtype, tag=nm, name=f"c_{nm}")
                nc.sync.dma_start(out=t[(slice(None),) * 2], in_=prm[nm][:, :])
                ct[nm] = t
            identb = cp.tile([P, P], bf16)
            nc.vector.tensor_copy(out=identb[:, :], in_=ct["identf"][:, :])

            dl_t = []
            for b in range(2):
                dlt = cp.tile([P, schs[b]["TB"]], bf16, tag=f"dl{b}", name=f"dl_t{b}")
                nc.sync.dma_start(out=dlt[:, :], in_=prm[f"dl{b}"][:, :])
                dl_t.append(dlt)
            dist_t, dist2_t = [], []
            for b in range(2):
                d1 = cp.tile([P, NW], f32, tag=f"dist{b}", name=f"dist_t{b}")
                nc.sync.dma_start(out=d1[:, :], in_=prm[f"dist{b}"][:, :])
                dist_t.append(d1)
                d2 = cp.tile([P, NW], f32, tag=f"dist2{b}", name=f"dist2_t{b}")
                nc.sync.dma_start(out=d2[:, :], in_=prm[f"dist2{b}"][:, :])
                dist2_t.append(d2)

            # =========== phase A: L1 table builds (both branches) ===========
            midp_ctx = tc.tile_pool(name="midp", bufs=1)
            midp = midp_ctx.__enter__()
            latep_ctx = tc.tile_pool(name="latep", bufs=1)
            latep = latep_ctx.__enter__()
            act1 = [midp.tile([P, NW * 32], bf16, tag=f"act1{b}", name=f"act1_{b}")
                    for b in range(2)]
            hself1 = [midp.tile([P, NW * 32], bf16, tag=f"hs1{b}", name=f"hself1_{b}")
                      for b in range(2)]
            hself2 = [None, None]
            act2 = [None, None]

            xtp_ctx = tc.tile_pool(name="xtp", bufs=1)
            xtp = xtp_ctx.__enter__()
            for b in range(2):
                xT = xtp.tile([P, CH, 8], bf16, tag="xT", name=f"xT_{b}")
                nc.sync.dma_start(out=xT[:, :, :], in_=prm[f"xT{b}"][:, :])
                hs1f = sm.tile([P, 32], f32, tag="hs1f")
                for w in range(NW):
                    hp = auxp.tile([P, 32], f32, tag="aux", space="PSUM")
                    nc.tensor.matmul(out=hp[:, :], lhsT=xT[:, 16 * w:16 * (w + 1), :],
                                     rhs=ct["W1pi"][:, :], start=True, stop=True)
                    nc.vector.tensor_scalar_mul(out=hs1f[:, :], in0=hp[:, :],
                                                scalar1=dist_t[b][:, w:w + 1])
                    nc.vector.tensor_add(
                        out=hself1[b][:, w * 32:(w + 1) * 32], in0=hs1f[:, :],
                        in1=ct["b1rep"][:, :])
                bnd = midp.tile([16, CH, 2], bf16, tag="bnd", name=f"bnd_{b}")
                NCH = 8
                cw = CH // NCH
                for g in range(8):
                    for u in range(2):
                        for chk in range(NCH):
                            c0 = chk * cw
                            bp = auxp.tile([16, cw], f32, tag="aux", space="PSUM")
                            nc.tensor.matmul(
                                out=bp[:, :],
                                lhsT=ct["W1eo"][:, 16 * u:16 * u + 16],
                                rhs=xT[:, c0:c0 + cw, g],
                                start=True, stop=True)
                            nc.vector.tensor_copy(
                                out=bnd[:, c0:c0 + cw, u], in_=bp[:, :])
                    nc.sync.dma_start(out=t1loc[b][16 * g:16 * g + 16, :],
                                      in_=bnd[:, :, :])
                nc.gpsimd.collective_compute(
                    "AllGather", mybir.AluOpType.bypass,
                    replica_groups=[list(range(NCORE))],
                    ins=[t1loc[b][:, :]], outs=[t1full[b][:, :]])

            # =========== gather/scatter machinery ===========
            def gather_layer(b, layer, tab, aggp, tpool, hself, act_out,
                             scale_out):
                sch = schs[b]
                L, NB, sched = sch["L"], sch["NB"], sch["sched"]
                wdiv, wmul = (16, 32) if layer == 1 else (32, 16)
                first_gen, last_win = {}, {}
                for t in range(NB * 8):
                    for (col, w) in sched[t]:
                        gen = w // wdiv
                        if gen not in first_gen:
                            first_gen[gen] = col
                        last_win[w] = col
                ncall = (L + KI - 1) // KI
                oh_state = {"c0": -99999, "tile": None}
                gen_tiles = {}

                def post_window(w):
                    gen = w // wdiv
                    off = (w % wdiv) * wmul
                    ag = gen_tiles[gen]
                    tmp = sm.tile([P, 32], f32, tag="post", name=f"post_{b}_{layer}_{w}")
                    nc.vector.tensor_scalar_mul(
                        out=tmp[:, 0:wmul], in0=ag[:, off:off + wmul],
                        scalar1=dist_t[b][:, w:w + 1])
                    nc.vector.tensor_add(out=tmp[:, 0:wmul], in0=tmp[:, 0:wmul],
                                         in1=hself[:, w * wmul:(w + 1) * wmul])
                    nc.vector.tensor_scalar_max(
                        out=tmp[:, 0:wmul], in0=tmp[:, 0:wmul], scalar1=0.0)
                    if scale_out:
                        nc.vector.tensor_scalar_mul(
                            out=act_out[:, w * wmul:(w + 1) * wmul],
                            in0=tmp[:, 0:wmul], scalar1=dist_t[b][:, w:w + 1])
                    else:
                        nc.vector.tensor_copy(
                            out=act_out[:, w * wmul:(w + 1) * wmul],
                            in_=tmp[:, 0:wmul])

                for ci in range(ncall):
                    ni = min(KI, L - ci * KI)
                    idc = sp.tile([P, KI // 16], i16, tag="idc", bufs=3,
                                  name=f"idc_{b}_{layer}_{ci}")
                    nc.sync.dma_start(
                        out=idc[:, 0:ni // 16],
                        in_=prm[f"idx{b}"][:, ci * (KI // 16):ci * (KI // 16) + ni // 16])
                    if layer == 1:
                        msgs = sp.tile([P, KI, 2], bf16, tag="msgs", bufs=3,
                                       name=f"msgs1_{b}_{ci}")
                        nc.gpsimd.ap_gather(
                            out_ap=msgs[:, 0:ni, :], in_ap=tab[:, :, :],
                            idxs_ap=idc[:, 0:ni // 16],
                            channels=P, num_elems=NT, d=2, num_idxs=ni)
                    else:
                        msgs = sp.tile([P, KI], f32, tag="msgs", bufs=3,
                                       name=f"msgs2_{b}_{ci}")
                        nc.gpsimd.ap_gather(
                            out_ap=msgs[:, 0:ni], in_ap=tab[:, :],
                            idxs_ap=idc[:, 0:ni // 16],
                            channels=P, num_elems=NT, d=1, num_idxs=ni)
                    for kk in range(ni // P):
                        tbase = (ci * (KI // P) + kk) * 8
                        if layer == 1:
                            tp = tpool.tile([P, 2, P], bf16, tag="tp", space="PSUM")
                            for u in range(2):
                                nc.tensor.transpose(
                                    out=tp[:, u, :],
                                    in_=msgs[:, kk * P:(kk + 1) * P, u],
                                    identity=identb[:, :])
                            rhsT = sm.tile([P, 2, P], bf16, tag="rhsT")
                            nc.vector.tensor_copy(out=rhsT[:, :, :], in_=tp[:, :, :])
                        else:
                            tp = tpool.tile([P, P], f32, tag="tp", space="PSUM")
                            nc.tensor.transpose(
                                out=tp[:, :], in_=msgs[:, kk * P:(kk + 1) * P],
                                identity=ct["identf"][:, :])
                            rhsT = sm.tile([P, 2, P], bf16, tag="rhsT")
                            nc.vector.tensor_copy(out=rhsT[:, 0, :], in_=tp[:, :])
                        done_wins = []
                        for q in range(8):
                            for (col, w) in sched[tbase + q]:
                                if not (oh_state["c0"] <= col < oh_state["c0"] + 32):
                                    c0 = (col // 32) * 32
                                    oh = sp.tile([P, 32, P], bf16, tag="oh",
                                                 name=f"oh_{b}_{layer}_{c0}")
                                    nc.vector.tensor_tensor(
                                        out=oh[:, :, :],
                                        in0=dl_t[b][:, c0:c0 + 32, None].to_broadcast([P, 32, P]),
                                        in1=ct["iota"][:, None, :].to_broadcast([P, 32, P]),
                                        op=mybir.AluOpType.is_equal)
                                    oh_state["c0"] = c0
                                    oh_state["tile"] = oh
                                ohc = oh_state["tile"]
                                gen = w // wdiv
                                if gen not in gen_tiles:
                                    gen_tiles[gen] = aggp.tile(
                                        [P, 512], f32, tag=f"agg{gen % 4}",
                                        name=f"agg_{b}_{layer}_{gen}", space="PSUM")
                                ag = gen_tiles[gen]
                                off = (w % wdiv) * wmul
                                st = (first_gen[gen] == col)
                                if layer == 1:
                                    nc.tensor.matmul(
                                        out=ag[:, off:off + 32],
                                        lhsT=ohc[:, col - oh_state["c0"], :],
                                        rhs=rhsT[:, :, 16 * q:16 * q + 16],
                                        start=st, stop=(last_win[w] == col),
                                        skip_group_check=True)
                                else:
                                    nc.tensor.matmul(
                                        out=ag[:, off:off + 16],
                                        lhsT=ohc[:, col - oh_state["c0"], :],
                                        rhs=rhsT[:, 0, 16 * q:16 * q + 16],
                                        start=st, stop=(last_win[w] == col),
                                        skip_group_check=True)
                                if last_win[w] == col:
                                    done_wins.append(w)
                        for w in done_wins:
                            post_window(w)

            def build_tab2(b, latep):
                act1T = xtp.tile([32, CH, 8], bf16, tag="xT", name=f"act1T_{b}")
                for w in range(NW):
                    ap_ = auxp.tile([32, P], bf16, tag="aux", space="PSUM")
                    nc.tensor.transpose(out=ap_[:, :],
                                        in_=act1[b][:, w * 32:(w + 1) * 32],
                                        identity=identb[:, :])
                    nc.vector.tensor_copy(out=act1T[:, 16 * w:16 * (w + 1), :],
                                          in_=ap_[:, :])
                hs2 = latep.tile([P, NW * 16], bf16, tag=f"hs2_{b}", name=f"hself2_{b}")
                hs2f = sm.tile([P, 16], f32, tag="hs2f")
                for w in range(NW):
                    hp = auxp.tile([P, 16], f32, tag="aux", space="PSUM")
                    nc.tensor.matmul(out=hp[:, :],
                                     lhsT=act1T[:, 16 * w:16 * (w + 1), :],
                                     rhs=ct["W2p"][:, :], start=True, stop=True)
                    nc.vector.tensor_scalar_mul(out=hs2f[:, :], in0=hp[:, :],
                                                scalar1=dist_t[b][:, w:w + 1])
                    nc.vector.tensor_add(
                        out=hs2[:, w * 16:(w + 1) * 16], in0=hs2f[:, :],
                        in1=ct["b2rep"][:, :])
                bnd2 = midp.tile([16, CH], f32, tag="bnd", name=f"bnd2_{b}")
                NCH = 8
                cw = CH // NCH
                for g in range(8):
                    for chk in range(NCH):
                        c0 = chk * cw
                        bp = auxp.tile([16, cw], f32, tag="aux", space="PSUM")
                        nc.tensor.matmul(
                            out=bp[:, :], lhsT=ct["W2p"][:, :],
                            rhs=act1T[:, c0:c0 + cw, g],
                            start=True, stop=True)
                        nc.vector.tensor_copy(
                            out=bnd2[:, c0:c0 + cw], in_=bp[:, :])
                    nc.sync.dma_start(out=t2loc[b][16 * g:16 * g + 16, :],
                                      in_=bnd2[:, :])
                nc.gpsimd.collective_compute(
                    "AllGather", mybir.AluOpType.bypass,
                    replica_groups=[list(range(NCORE))],
                    ins=[t2loc[b][:, :]], outs=[t2full[b][:, :]])
                return hs2

            # =========== phase B: L1 gathers + L2 table builds ===========
            l1p_ctx = tc.tile_pool(name="l1p", bufs=1, space="PSUM")
            l1p = l1p_ctx.__enter__()
            tp1_ctx = tc.tile_pool(name="tp1p", bufs=2, space="PSUM")
            tp1p = tp1_ctx.__enter__()
            for b in range(2):
                tab1 = tbp.tile([P, NT, 2], bf16, tag="tabfull", name=f"tab1_{b}")
                for c in range(NCORE):
                    nc.sync.dma_start(out=tab1[:, c * CH:(c + 1) * CH, :],
                                      in_=t1full[b][c * P:(c + 1) * P, :])
                gather_layer(b, 1, tab1, l1p, tp1p, hself1[b], act1[b],
                             scale_out=True)
                hself2[b] = build_tab2(b, latep)
            xtp_ctx.__exit__(None, None, None)
            tp1_ctx.__exit__(None, None, None)
            l1p_ctx.__exit__(None, None, None)

            # =========== phase C: L2 gathers + pool partials ===========
            dtp_ctx = tc.tile_pool(name="dtp", bufs=1)
            dtp = dtp_ctx.__enter__()
            l2p_ctx = tc.tile_pool(name="l2p", bufs=1, space="PSUM")
            l2p = l2p_ctx.__enter__()
            tp2_ctx = tc.tile_pool(name="tp2p", bufs=2, space="PSUM")
            tp2p = tp2_ctx.__enter__()
            for b in range(2):
                tab2 = tbp.tile([P, NT], f32, tag="tabfull", name=f"tab2_{b}")
                for c in range(NCORE):
                    nc.sync.dma_start(out=tab2[:, c * CH:(c + 1) * CH],
                                      in_=t2full[b][c * P:(c + 1) * P, :])
                a2 = latep.tile([P, NW * 16], bf16, tag=f"act2{b}", name=f"act2_{b}")
                act2[b] = a2
                gather_layer(b, 2, tab2, l2p, tp2p, hself2[b], a2,
                             scale_out=False)
                DtT = dtp.tile([P, NW * G], bf16, tag="DtT", name=f"DtT_{b}")
                nc.sync.dma_start(out=DtT[:, :], in_=prm[f"DtT{b}"][:, :])
                pp = auxp.tile([G, 16], f32, tag="aux", name=f"poolp_{b}",
                               space="PSUM")
                for w in range(NW):
                    nc.tensor.matmul(out=pp[:, :],
                                     lhsT=DtT[:, w * G:(w + 1) * G],
                                     rhs=a2[:, w * 16:(w + 1) * 16],
                                     start=(w == 0), stop=(w == NW - 1))
                ps = sm.tile([G, 16], f32, tag="pools")
                nc.vector.tensor_copy(out=ps[:, :], in_=pp[:, :])
                nc.sync.dma_start(out=pool_in[:, 16 * b:16 * (b + 1)], in_=ps[:, :])
            tp2_ctx.__exit__(None, None, None)
            l2p_ctx.__exit__(None, None, None)
            dtp_ctx.__exit__(None, None, None)

            # =========== tail: AllReduce + pool scale + W3 + MLP ===========
            nc.gpsimd.collective_compute(
                "AllReduce", mybir.AluOpType.add,
                replica_groups=[list(range(NCORE))],
                ins=[pool_in[:, :]], outs=[pool_out[:, :]])
            pr = sm.tile([G, 32], f32, tag="pr")
            nc.sync.dma_start(out=pr[:, :], in_=pool_out[:, :])
            pm_ctx = tc.tile_pool(name="pm", bufs=1, space="PSUM")
            pm = pm_ctx.__enter__()
            pooled_cat = sm.tile([G, 16], f32, tag="pcat")
            for b in range(2):
                ic_t = sm.tile([G, 1], f32, tag="ic")
                nc.sync.dma_start(out=ic_t[:, :], in_=prm[f"ic{b}"][:, :])
                pb = sm.tile([G, 16], f32, tag="pb")
                nc.vector.tensor_scalar_mul(out=pb[:, :], in0=pr[:, 16 * b:16 * (b + 1)],
                                            scalar1=ic_t[:, :])
                pbT_p = pm.tile([16, G], f32, tag="pbT", name=f"pbT_{b}", space="PSUM")
                nc.tensor.transpose(out=pbT_p[:, :], in_=pb[:, :],
                                    identity=ct["identf"][0:G, 0:G])
                pbT = sm.tile([16, G], f32, tag="pbTs")
                nc.vector.tensor_copy(out=pbT[:, :], in_=pbT_p[:, :])
                po_p = pm.tile([G, 8], f32, tag="po", name=f"po_{b}", space="PSUM")
                nc.tensor.matmul(out=po_p[:, :], lhsT=pbT[:, :], rhs=ct["W3"][:, :],
                                 start=True, stop=True)
                nc.vector.tensor_add(out=pooled_cat[:, 8 * b:8 * (b + 1)],
                                     in0=po_p[:, :], in1=ct["b3r"][:, :])
            pcT_p = pm.tile([16, G], f32, tag="pcT", space="PSUM")
            nc.tensor.transpose(out=pcT_p[:, :], in_=pooled_cat[:, :],
                                identity=ct["identf"][0:G, 0:G])
            pcT = sm.tile([16, G], f32, tag="pcTs")
            nc.vector.tensor_copy(out=pcT[:, :], in_=pcT_p[:, :])
            m1_p = pm.tile([G, 8], f32, tag="m1", space="PSUM")
            nc.tensor.matmul(out=m1_p[:, :], lhsT=pcT[:, :], rhs=ct["mW1"][:, :],
                             start=True, stop=True)
            m1_s = sm.tile([G, 8], f32, tag="m1s")
            nc.vector.tensor_add(out=m1_s[:, :], in0=m1_p[:, :], in1=ct["mb1r"][:, :])
            nc.vector.tensor_scalar_max(out=m1_s[:, :], in0=m1_s[:, :], scalar1=0.0)
            m1T_p = pm.tile([8, G], f32, tag="m1T", space="PSUM")
            nc.tensor.transpose(out=m1T_p[:, :], in_=m1_s[:, :],
                                identity=ct["identf"][0:G, 0:G])
            m1T = sm.tile([8, G], f32, tag="m1Ts")
            nc.vector.tensor_copy(out=m1T[:, :], in_=m1T_p[:, :])
            m2_p = pm.tile([G, 2], f32, tag="m2", space="PSUM")
            nc.tensor.matmul(out=m2_p[:, :], lhsT=m1T[:, :], rhs=ct["mW2"][:, :],
                             start=True, stop=True)
            m2_s = sm.tile([G, 2], f32, tag="m2s")
            nc.vector.tensor_add(out=m2_s[:, :], in0=m2_p[:, :], in1=ct["mb2r"][:, :])
            nc.sync.dma_start(out=out_p[:, :], in_=m2_s[:, :])
            pm_ctx.__exit__(None, None, None)

            latep_ctx.__exit__(None, None, None)
            midp_ctx.__exit__(None, None, None)

    nc.compile()
    return nc


# --------------------------------------------------------------------------
# driver
# --------------------------------------------------------------------------

def _run(inputs, trace=False):
    global last_results
    x = [np.asarray(inputs["x0"], np.float32), np.asarray(inputs["x1"], np.float32)]
    ei = [np.asarray(inputs["edge_index0"]), np.asarray(inputs["edge_index1"])]
    bt = [np.asarray(inputs["batch0"]), np.asarray(inputs["batch1"])]

    schs = [_prep_branch(ei[b], bt[b]) for b in range(2)]

    W1 = np.asarray(inputs["W1"], np.float32)
    b1 = np.asarray(inputs["b1"], np.float32)
    W2 = np.asarray(inputs["W2"], np.float32)
    b2 = np.asarray(inputs["b2"], np.float32)
    W1eo = np.concatenate([W1[:, 0::2], W1[:, 1::2]], axis=1).astype(bfloat16)
    common = dict(
        W1eo=W1eo,
        W1pi=W1[:, PI].astype(bfloat16),
        b1rep=np.broadcast_to(b1[PI], (P, 32)).astype(np.float32).copy(),
        W2p=np.asarray(W2[PI, :], np.float32).astype(bfloat16),
        b2rep=np.broadcast_to(b2, (P, 16)).astype(np.float32).copy(),
        W3=np.asarray(inputs["W3"], np.float32),
        b3r=np.broadcast_to(np.asarray(inputs["b3"], np.float32), (G, 8)).copy(),
        mW1=np.asarray(inputs["mW1"], np.float32),
        mb1r=np.broadcast_to(np.asarray(inputs["mb1"], np.float32), (G, 8)).copy(),
        mW2=np.asarray(inputs["mW2"], np.float32),
        mb2r=np.broadcast_to(np.asarray(inputs["mb2"], np.float32), (G, 2)).copy(),
        identf=np.eye(P, dtype=np.float32),
        iota=np.ascontiguousarray(
            np.broadcast_to(np.arange(P, dtype=np.float32), (P, P))).astype(bfloat16),
        ic0=schs[0]["inv_cnt"], ic1=schs[1]["inv_cnt"],
    )

    xpad = []
    diss = []
    for b in range(2):
        t = np.zeros((NPAD, 128), np.float32)
        t[:N] = x[b]
        xpad.append(t)
        diss.append(schs[b]["dis"])

    in_maps = []
    for c in range(NCORE):
        m = dict(common)
        for b in range(2):
            pc = schs[b]["per_core"][c]
            m[f"xT{b}"] = np.ascontiguousarray(
                (xpad[b][c * SH:(c + 1) * SH]
                 * diss[b][c * SH:(c + 1) * SH, None]).T).astype(bfloat16)
            m[f"idx{b}"] = pc["idx"]
            m[f"dl{b}"] = pc["dl"]
            m[f"dist{b}"] = pc["dist"]
            m[f"dist2{b}"] = pc["dist2"]
            m[f"DtT{b}"] = pc["DtT"]
        in_maps.append(m)

    nc = _build_program(schs)
    res = run_bass_kernel_spmd(nc, in_maps, list(range(NCORE)), trace=trace)
    last_results = res
    return np.asarray(res.results[0]["out"], np.float32)


def kernel(**inputs):
    return _run(inputs, trace=False)


# revision 15
# speedup vs baseline: 1.1938x; 1.1938x over previous
"""Trainium2 Bass kernel for the BGNN (3-layer GCN x 2 branches + mean-pool + MLP).

v2 design (ap_gather-based):
  - Nodes dst-sharded across 8 cores (SH=12544/core). Per branch, edges split
    into 8 gather streams by src%8; each GpSimd Q7 core gathers its stream's
    source features from an SBUF-resident replicated table via ap_gather
    (features transposed onto partitions, 8-node column packing).
  - L1 table bf16 feat-pairs in u32 units ([128, NT, 2] bf16); L2 table f32
    [128, NT]. Tables device-built per band, AllGathered, reloaded packed.
  - Per 128-edge-slot block: TensorE strided-plane transposes flip [feat,edge]
    to [edge,feat]; dl-vs-iota one-hot matmuls scatter into per-window PSUM
    accumulators (bank-first start, bank-last stop).
  - Layer 3 + mean-pool folded into host-precomputed structural matrix
    Dt[s,g]: pool = ((Dt^T @ act2)/cnt) @ W3 + b3.  MLP replicated per core.
  - Block->window schedule baked into the SPMD program: per-(stream,window)
    run capacities common across cores (max over cores, padded to 32);
    window-straddling blocks use multiple masked one-hot fragments.
"""
import sys

sys.path.insert(0, "/opt/trn_rl_repo")

import numpy as np
import ml_dtypes

import concourse.bacc as bacc
import concourse.bass as bass
import concourse.mybir as mybir
import concourse.tile as tile
from concourse.bass_utils import run_bass_kernel_spmd

P = 128
NCORE = 8
G = 64
N = 100000
SH = 12544
NPAD = SH * NCORE
NW = SH // P                # 98
NT = NPAD // 8              # 12544
CH = SH // 8                # 1568
KI = 4096
PI = np.concatenate([np.arange(0, 32, 2), np.arange(1, 32, 2)])

bfloat16 = ml_dtypes.bfloat16
last_results = None


def _pad_to(x, m):
    return (x + m - 1) // m * m


# --------------------------------------------------------------------------
# host prep
# --------------------------------------------------------------------------

def _prep_branch(ei, batch):
    src = ei[0].astype(np.int64)
    dst = ei[1].astype(np.int64)
    deg = np.bincount(dst, minlength=N).astype(np.float32) + 1.0
    dis = np.ones(NPAD, np.float32)
    dis[:N] = deg ** -0.5

    bpad = np.zeros(NPAD, np.int64)
    bpad[:N] = batch.astype(np.int64)
    flat = np.bincount(src * G + bpad[dst], weights=dis[dst].astype(np.float64),
                      minlength=NPAD * G)
    Dt = flat.reshape(NPAD, G).astype(np.float32)
    Dt *= dis[:, None]
    Dt[np.arange(N), bpad[:N]] += dis[:N] ** 2
    cnt = np.bincount(batch.astype(np.int64), minlength=G).astype(np.float32)
    inv_cnt = (1.0 / np.maximum(cnt, 1.0)).reshape(G, 1)

    core = dst // SH
    loc_s = src % SH
    stream = loc_s % 8
    gidx = (src // SH) * CH + loc_s // 8
    win = (dst % SH) // P
    dlv = (dst % P).astype(np.float32)

    key = (core * 8 + stream) * NW + win
    counts = np.bincount(key, minlength=NCORE * 8 * NW).reshape(NCORE, 8, NW)
    caps = _pad_to(counts.max(axis=0), 4)
    caps[0] = np.maximum(caps[0], 4)
    L = int(_pad_to(caps.sum(axis=1).max(), KI // 8 if False else P))
    NB = L // P

    starts = np.zeros((8, NW), np.int64)
    for q in range(8):
        np.cumsum(caps[q][:-1], out=starts[q][1:])

    # common block schedule
    frag_win = [[] for _ in range(NB * 8)]
    for q in range(8):
        for w in range(NW):
            s0, s1 = starts[q][w], starts[q][w] + caps[q][w]
            for k in range(s0 // P, min((s1 + P - 1) // P, NB)):
                frag_win[k * 8 + q].append(w)
    dlcols = []
    sched = []
    for t in range(NB * 8):
        kk, q = t // 8, t % 8
        ent = []
        for w in frag_win[t]:
            col = len(dlcols)
            s0, s1 = starts[q][w], starts[q][w] + caps[q][w]
            lo, hi = max(s0, kk * P), min(s1, (kk + 1) * P)
            dlcols.append((t, q, w, lo, hi))
            ent.append((col, w))
        sched.append(ent)
    TB = _pad_to(len(dlcols), 32)

    order = np.lexsort((win, stream, core))
    gidx_o, dl_o, win_o = gidx[order], dlv[order], win[order]
    core_o, stream_o = core[order], stream[order]
    cbounds = np.searchsorted(core_o, np.arange(NCORE + 1))
    per_core = []
    for c in range(NCORE):
        lo, hi = cbounds[c], cbounds[c + 1]
        gq, gw = stream_o[lo:hi], win_o[lo:hi]
        gi, gd = gidx_o[lo:hi], dl_o[lo:hi]
        idx_arr = np.zeros((8, L), np.int64)
        dl_full = np.full((8, NB * P), -1.0, np.float32)
        qb = np.searchsorted(gq, np.arange(9))
        for q in range(8):
            ql, qh = qb[q], qb[q + 1]
            wq, iq, dq = gw[ql:qh], gi[ql:qh], gd[ql:qh]
            wcnt = np.bincount(wq, minlength=NW)
            wstart = np.zeros(NW, np.int64)
            np.cumsum(wcnt[:-1], out=wstart[1:])
            pos = starts[q][wq] + (np.arange(qh - ql) - wstart[wq])
            idx_arr[q, pos] = iq
            dl_full[q, pos] = dq
        idx_tile = np.zeros((P, L // 16), np.int16)
        for g in range(8):
            idx_tile[16 * g:16 * g + 16, :] = \
                idx_arr[g].astype(np.int16).reshape(L // 16, 16).T
        dl_tile = np.full((P, TB), -1.0, np.float32)
        for col, (t, q, w, flo, fhi) in enumerate(dlcols):
            kk = t // 8
            seg = dl_full[q, kk * P:(kk + 1) * P].copy()
            mask = np.zeros(P, bool)
            mask[flo - kk * P:fhi - kk * P] = True
            seg[~mask] = -1.0
            dl_tile[:, col] = seg
        dis_t = np.ascontiguousarray(dis[c * SH:(c + 1) * SH].reshape(NW, P).T)
        disP = np.ascontiguousarray(dis[c * SH:(c + 1) * SH].reshape(CH, 8).T)
        DtT = np.ascontiguousarray(
            Dt[c * SH:(c + 1) * SH].reshape(NW, P, G).transpose(1, 0, 2)
            .reshape(P, NW * G)).astype(bfloat16)
        per_core.append(dict(idx=idx_tile, dl=dl_tile.astype(bfloat16),
                             dist=dis_t, dist2=dis_t * dis_t, disP=disP,
                             DtT=DtT))
    return dict(sched=sched, TB=TB, NB=NB, L=L, inv_cnt=inv_cnt,
                per_core=per_core, dis=dis)


# --------------------------------------------------------------------------
# device program
# --------------------------------------------------------------------------

def _build_program(schs):
    nc = bacc.Bacc()
    f32 = mybir.dt.float32
    bf16 = mybir.dt.bfloat16
    i16 = mybir.dt.int16

    prm = {}
    for b in (0, 1):
        sch = schs[b]
        prm[f"xT{b}"] = nc.declare_dram_parameter(f"xT{b}", [P, SH], bf16, isOutput=False)
        prm[f"idx{b}"] = nc.declare_dram_parameter(f"idx{b}", [P, sch["L"] // 16], i16, isOutput=False)
        prm[f"dl{b}"] = nc.declare_dram_parameter(f"dl{b}", [P, sch["TB"]], bf16, isOutput=False)
        prm[f"dist{b}"] = nc.declare_dram_parameter(f"dist{b}", [P, NW], f32, isOutput=False)
        prm[f"dist2{b}"] = nc.declare_dram_parameter(f"dist2{b}", [P, NW], f32, isOutput=False)
        prm[f"DtT{b}"] = nc.declare_dram_parameter(f"DtT{b}", [P, NW * G], bf16, isOutput=False)
        prm[f"ic{b}"] = nc.declare_dram_parameter(f"ic{b}", [G, 1], f32, isOutput=False)
    for nm, shp, dt in (
        ("W1eo", [P, 32], bf16), ("W1pi", [P, 32], bf16), ("b1rep", [P, 32], f32),
        ("W2p", [32, 16], bf16), ("b2rep", [P, 16], f32),
        ("W3", [16, 8], f32), ("b3r", [G, 8], f32),
        ("mW1", [16, 8], f32), ("mb1r", [G, 8], f32),
        ("mW2", [8, 2], f32), ("mb2r", [G, 2], f32),
        ("identf", [P, P], f32), ("iota", [P, P], bf16),
    ):
        prm[nm] = nc.declare_dram_parameter(nm, shp, dt, isOutput=False)
    out_p = nc.declare_dram_parameter("out", [G, 2], f32, isOutput=True)

    t1loc = [nc.dram_tensor(f"t1loc{b}", [P, 2 * CH], bf16) for b in (0, 1)]
    t1full = [nc.dram_tensor(f"t1full{b}", [NCORE * P, 2 * CH], bf16) for b in (0, 1)]
    t2loc = [nc.dram_tensor(f"t2loc{b}", [P, CH], f32) for b in (0, 1)]
    t2full = [nc.dram_tensor(f"t2full{b}", [NCORE * P, CH], f32) for b in (0, 1)]
    pool_in = nc.dram_tensor("pool_in", [G, 32], f32)
    pool_out = nc.dram_tensor("pool_out", [G, 32], f32)

    with tile.TileContext(nc) as tc:
        with (
            tc.tile_pool(name="const", bufs=1) as cp,
            tc.tile_pool(name="tabs", bufs=1) as tbp,
            tc.tile_pool(name="stream", bufs=2) as sp,
            tc.tile_pool(name="small", bufs=3) as sm,
            tc.tile_pool(name="auxp", bufs=1, space="PSUM") as auxp,
        ):
            ct = {}
            for nm in ("W1eo", "W1pi", "b1rep", "W2p", "b2rep", "W3", "b3r",
                       "mW1", "mb1r", "mW2", "mb2r", "identf", "iota"):
                t = cp.tile(list(prm[nm].shape), prm[nm].dtype, tag=nm, name=f"c_{nm}")
                nc.sync.dma_start(out=t[(slice(None),) * 2], in_=prm[nm][:, :])
                ct[nm] = t
            identb = cp.tile([P, P], bf16)
            nc.vector.tensor_copy(out=identb[:, :], in_=ct["identf"][:, :])

            dl_t = []
            for b in range(2):
                dlt = cp.tile([P, schs[b]["TB"]], bf16, tag=f"dl{b}", name=f"dl_t{b}")
                nc.sync.dma_start(out=dlt[:, :], in_=prm[f"dl{b}"][:, :])
                dl_t.append(dlt)
            dist_t, dist2_t = [], []
            for b in range(2):
                d1 = cp.tile([P, NW], f32, tag=f"dist{b}", name=f"dist_t{b}")
                nc.sync.dma_start(out=d1[:, :], in_=prm[f"dist{b}"][:, :])
                dist_t.append(d1)
                d2 = cp.tile([P, NW], f32, tag=f"dist2{b}", name=f"dist2_t{b}")
                nc.sync.dma_start(out=d2[:, :], in_=prm[f"dist2{b}"][:, :])
                dist2_t.append(d2)

            # =========== phase A: L1 table builds (both branches) ===========
            midp_ctx = tc.tile_pool(name="midp", bufs=1)
            midp = midp_ctx.__enter__()
            latep_ctx = tc.tile_pool(name="latep", bufs=1)
            latep = latep_ctx.__enter__()
            act1 = [midp.tile([P, NW * 32], bf16, tag=f"act1{b}", name=f"act1_{b}")
                    for b in range(2)]
            hself1 = [midp.tile([P, NW * 32], bf16, tag=f"hs1{b}", name=f"hself1_{b}")
                      for b in range(2)]
            hself2 = [None, None]
            act2 = [None, None]

            xtp_ctx = tc.tile_pool(name="xtp", bufs=1)
            xtp = xtp_ctx.__enter__()
            for b in range(2):
                xT = xtp.tile([P, CH, 8], bf16, tag="xT", name=f"xT_{b}")
                nc.sync.dma_start(out=xT[:, :, :], in_=prm[f"xT{b}"][:, :])
                hs1f = sm.tile([P, 32], f32, tag="hs1f")
                for w in range(NW):
                    hp = auxp.tile([P, 32], f32, tag="aux", space="PSUM")
                    nc.tensor.matmul(out=hp[:, :], lhsT=xT[:, 16 * w:16 * (w + 1), :],
                                     rhs=ct["W1pi"][:, :], start=True, stop=True)
                    nc.vector.tensor_scalar_mul(out=hs1f[:, :], in0=hp[:, :],
                                                scalar1=dist_t[b][:, w:w + 1])
                    nc.vector.tensor_add(
                        out=hself1[b][:, w * 32:(w + 1) * 32], in0=hs1f[:, :],
                        in1=ct["b1rep"][:, :])
                bnd = midp.tile([16, CH, 2], bf16, tag="bnd", name=f"bnd_{b}")
                NCH = 8
                cw = CH // NCH
                for g in range(8):
                    for u in range(2):
                        for chk in range(NCH):
                            c0 = chk * cw
                            bp = auxp.tile([16, cw], f32, tag="aux", space="PSUM")
                            nc.tensor.matmul(
                                out=bp[:, :],
                                lhsT=ct["W1eo"][:, 16 * u:16 * u + 16],
                                rhs=xT[:, c0:c0 + cw, g],
                                start=True, stop=True)
                            nc.vector.tensor_copy(
                                out=bnd[:, c0:c0 + cw, u], in_=bp[:, :])
                    nc.sync.dma_start(out=t1loc[b][16 * g:16 * g + 16, :],
                                      in_=bnd[:, :, :])
                nc.gpsimd.collective_compute(
                    "AllGather", mybir.AluOpType.bypass,
                    replica_groups=[list(range(NCORE))],
                    ins=[t1loc[b][:, :]], outs=[t1full[b][:, :]])

            # =========== gather/scatter machinery ===========
            def gather_layer(b, layer, tab, aggp, tpool, hself, act_out,
                             scale_out):
                sch = schs[b]
                L, NB, sched = sch["L"], sch["NB"], sch["sched"]
                wdiv, wmul = (16, 32) if layer == 1 else (32, 16)
                first_gen, last_win = {}, {}
                for t in range(NB * 8):
                    for (col, w) in sched[t]:
                        gen = w // wdiv
                        if gen not in first_gen:
                            first_gen[gen] = col
                        last_win[w] = col
                ncall = (L + KI - 1) // KI
                oh_state = {"c0": -99999, "tile": None}
                gen_tiles = {}

                def post_window(w):
                    gen = w // wdiv
                    off = (w % wdiv) * wmul
                    ag = gen_tiles[gen]
                    tmp = sm.tile([P, 32], f32, tag="post", name=f"post_{b}_{layer}_{w}")
                    nc.vector.tensor_scalar_mul(
                        out=tmp[:, 0:wmul], in0=ag[:, off:off + wmul],
                        scalar1=dist_t[b][:, w:w + 1])
                    nc.vector.tensor_add(out=tmp[:, 0:wmul], in0=tmp[:, 0:wmul],
                                         in1=hself[:, w * wmul:(w + 1) * wmul])
                    nc.vector.tensor_scalar_max(
                        out=tmp[:, 0:wmul], in0=tmp[:, 0:wmul], scalar1=0.0)
                    if scale_out:
                        nc.vector.tensor_scalar_mul(
                            out=act_out[:, w * wmul:(w + 1) * wmul],
                            in0=tmp[:, 0:wmul], scalar1=dist_t[b][:, w:w + 1])
                    else:
                        nc.vector.tensor_copy(
                            out=act_out[:, w * wmul:(w + 1) * wmul],
                            in_=tmp[:, 0:wmul])

                for ci in range(ncall):
                    ni = min(KI, L - ci * KI)
                    idc = sp.tile([P, KI // 16], i16, tag="idc", bufs=3,
                                  name=f"idc_{b}_{layer}_{ci}")
                    nc.sync.dma_start(
                        out=idc[:, 0:ni // 16],
                        in_=prm[f"idx{b}"][:, ci * (KI // 16):ci * (KI // 16) + ni // 16])
                    if layer == 1:
                        msgs = sp.tile([P, KI, 2], bf16, tag="msgs",
                                       name=f"msgs1_{b}_{ci}")
                        nc.gpsimd.ap_gather(
                            out_ap=msgs[:, 0:ni, :], in_ap=tab[:, :, :],
                            idxs_ap=idc[:, 0:ni // 16],
                            channels=P, num_elems=NT, d=2, num_idxs=ni)
                    else:
                        msgs = sp.tile([P, KI], f32, tag="msgs",
                                       name=f"msgs2_{b}_{ci}")
                        nc.gpsimd.ap_gather(
                            out_ap=msgs[:, 0:ni], in_ap=tab[:, :],
                            idxs_ap=idc[:, 0:ni // 16],
                            channels=P, num_elems=NT, d=1, num_idxs=ni)
                    for kk in range(ni // P):
                        tbase = (ci * (KI // P) + kk) * 8
                        if layer == 1:
                            tp = tpool.tile([P, 2, P], bf16, tag="tp", space="PSUM")
                            for u in range(2):
                                nc.tensor.transpose(
                                    out=tp[:, u, :],
                                    in_=msgs[:, kk * P:(kk + 1) * P, u],
                                    identity=identb[:, :])
                            rhsT = sm.tile([P, 2, P], bf16, tag="rhsT")
                            nc.vector.tensor_copy(out=rhsT[:, :, :], in_=tp[:, :, :])
                        else:
                            tp = tpool.tile([P, P], f32, tag="tp", space="PSUM")
                            nc.tensor.transpose(
                                out=tp[:, :], in_=msgs[:, kk * P:(kk + 1) * P],
                                identity=ct["identf"][:, :])
                            rhsT = sm.tile([P, 2, P], bf16, tag="rhsT")
                            nc.vector.tensor_copy(out=rhsT[:, 0, :], in_=tp[:, :])
                        done_wins = []
                        for q in range(8):
                            for (col, w) in sched[tbase + q]:
                                if not (oh_state["c0"] <= col < oh_state["c0"] + 32):
                                    c0 = (col // 32) * 32
                                    oh = sp.tile([P, 32, P], bf16, tag="oh",
                                                 name=f"oh_{b}_{layer}_{c0}")
                                    nc.vector.tensor_tensor(
                                        out=oh[:, :, :],
                                        in0=dl_t[b][:, c0:c0 + 32, None].to_broadcast([P, 32, P]),
                                        in1=ct["iota"][:, None, :].to_broadcast([P, 32, P]),
                                        op=mybir.AluOpType.is_equal)
                                    oh_state["c0"] = c0
                                    oh_state["tile"] = oh
                                ohc = oh_state["tile"]
                                gen = w // wdiv
                                if gen not in gen_tiles:
                                    gen_tiles[gen] = aggp.tile(
                                        [P, 512], f32, tag=f"agg{gen % 4}",
                                        name=f"agg_{b}_{layer}_{gen}", space="PSUM")
                                ag = gen_tiles[gen]
                                off = (w % wdiv) * wmul
                                st = (first_gen[gen] == col)
                                if layer == 1:
                                    nc.tensor.matmul(
                                        out=ag[:, off:off + 32],
                                        lhsT=ohc[:, col - oh_state["c0"], :],
                                        rhs=rhsT[:, :, 16 * q:16 * q + 16],
                                        start=st, stop=(last_win[w] == col),
                                        skip_group_check=True)
                                else:
                                    nc.tensor.matmul(
                                        out=ag[:, off:off + 16],
                                        lhsT=ohc[:, col - oh_state["c0"], :],
                                        rhs=rhsT[:, 0, 16 * q:16 * q + 16],
                                        start=st, stop=(last_win[w] == col),
                                        skip_group_check=True)
                                if last_win[w] == col:
                                    done_wins.append(w)
                        for w in done_wins:
                            post_window(w)

            def build_tab2(b, latep):
                act1T = xtp.tile([32, CH, 8], bf16, tag="xT", name=f"act1T_{b}")
                for w in range(NW):
                    ap_ = auxp.tile([32, P], bf16, tag="aux", space="PSUM")
                    nc.tensor.transpose(out=ap_[:, :],
                                        in_=act1[b][:, w * 32:(w + 1) * 32],
                                        identity=identb[:, :])
                    nc.vector.tensor_copy(out=act1T[:, 16 * w:16 * (w + 1), :],
                                          in_=ap_[:, :])
                hs2 = latep.tile([P, NW * 16], bf16, tag=f"hs2_{b}", name=f"hself2_{b}")
                hs2f = sm.tile([P, 16], f32, tag="hs2f")
                for w in range(NW):
                    hp = auxp.tile([P, 16], f32, tag="aux", space="PSUM")
                    nc.tensor.matmul(out=hp[:, :],
                                     lhsT=act1T[:, 16 * w:16 * (w + 1), :],
                                     rhs=ct["W2p"][:, :], start=True, stop=True)
                    nc.vector.tensor_scalar_mul(out=hs2f[:, :], in0=hp[:, :],
                                                scalar1=dist_t[b][:, w:w + 1])
                    nc.vector.tensor_add(
                        out=hs2[:, w * 16:(w + 1) * 16], in0=hs2f[:, :],
                        in1=ct["b2rep"][:, :])
                bnd2 = midp.tile([16, CH], f32, tag="bnd", name=f"bnd2_{b}")
                NCH = 8
                cw = CH // NCH
                for g in range(8):
                    for chk in range(NCH):
                        c0 = chk * cw
                        bp = auxp.tile([16, cw], f32, tag="aux", space="PSUM")
                        nc.tensor.matmul(
                            out=bp[:, :], lhsT=ct["W2p"][:, :],
                            rhs=act1T[:, c0:c0 + cw, g],
                            start=True, stop=True)
                        nc.vector.tensor_copy(
                            out=bnd2[:, c0:c0 + cw], in_=bp[:, :])
                    nc.sync.dma_start(out=t2loc[b][16 * g:16 * g + 16, :],
                                      in_=bnd2[:, :])
                nc.gpsimd.collective_compute(
                    "AllGather", mybir.AluOpType.bypass,
                    replica_groups=[list(range(NCORE))],
                    ins=[t2loc[b][:, :]], outs=[t2full[b][:, :]])
                return hs2

            # =========== phase B: L1 gathers + L2 table builds ===========
            l1p_ctx = tc.tile_pool(name="l1p", bufs=1, space="PSUM")
            l1p = l1p_ctx.__enter__()
            tp1_ctx = tc.tile_pool(name="tp1p", bufs=2, space="PSUM")
            tp1p = tp1_ctx.__enter__()
            for b in range(2):
                tab1 = tbp.tile([P, NT, 2], bf16, tag="tabfull", name=f"tab1_{b}")
                for c in range(NCORE):
                    nc.sync.dma_start(out=tab1[:, c * CH:(c + 1) * CH, :],
                                      in_=t1full[b][c * P:(c + 1) * P, :])
                gather_layer(b, 1, tab1, l1p, tp1p, hself1[b], act1[b],
                             scale_out=True)
                hself2[b] = build_tab2(b, latep)
            xtp_ctx.__exit__(None, None, None)
            tp1_ctx.__exit__(None, None, None)
            l1p_ctx.__exit__(None, None, None)

            # =========== phase C: L2 gathers + pool partials ===========
            dtp_ctx = tc.tile_pool(name="dtp", bufs=1)
            dtp = dtp_ctx.__enter__()
            l2p_ctx = tc.tile_pool(name="l2p", bufs=1, space="PSUM")
            l2p = l2p_ctx.__enter__()
            tp2_ctx = tc.tile_pool(name="tp2p", bufs=2, space="PSUM")
            tp2p = tp2_ctx.__enter__()
            for b in range(2):
                tab2 = tbp.tile([P, NT], f32, tag="tabfull", name=f"tab2_{b}")
                for c in range(NCORE):
                    nc.sync.dma_start(out=tab2[:, c * CH:(c + 1) * CH],
                                      in_=t2full[b][c * P:(c + 1) * P, :])
                a2 = latep.tile([P, NW * 16], bf16, tag=f"act2{b}", name=f"act2_{b}")
                act2[b] = a2
                gather_layer(b, 2, tab2, l2p, tp2p, hself2[b], a2,
                             scale_out=False)
                DtT = dtp.tile([P, NW * G], bf16, tag="DtT", name=f"DtT_{b}")
                nc.sync.dma_start(out=DtT[:, :], in_=prm[f"DtT{b}"][:, :])
                pp = auxp.tile([G, 16], f32, tag="aux", name=f"poolp_{b}",
                               space="PSUM")
                for w in range(NW):
                    nc.tensor.matmul(out=pp[:, :],
                                     lhsT=DtT[:, w * G:(w + 1) * G],
                                     rhs=a2[:, w * 16:(w + 1) * 16],
                                     start=(w == 0), stop=(w == NW - 1))
                ps = sm.tile([G, 16], f32, tag="pools")
                nc.vector.tensor_copy(out=ps[:, :], in_=pp[:, :])
                nc.sync.dma_start(out=pool_in[:, 16 * b:16 * (b + 1)], in_=ps[:, :])
            tp2_ctx.__exit__(None, None, None)
            l2p_ctx.__exit__(None, None, None)
            dtp_ctx.__exit__(None, None, None)

            # =========== tail: AllReduce + pool scale + W3 + MLP ===========
            nc.gpsimd.collective_compute(
                "AllReduce", mybir.AluOpType.add,
                replica_groups=[list(range(NCORE))],
                ins=[pool_in[:, :]], outs=[pool_out[:, :]])
            pr = sm.tile([G, 32], f32, tag="pr")
            nc.sync.dma_start(out=pr[:, :], in_=pool_out[:, :])
            pm_ctx = tc.tile_pool(name="pm", bufs=1, space="PSUM")
            pm = pm_ctx.__enter__()
            pooled_cat = sm.tile([G, 16], f32, tag="pcat")
            for b in range(2):
                ic_t = sm.tile([G, 1], f32, tag="ic")
                nc.sync.dma_start(out=ic_t[:, :], in_=prm[f"ic{b}"][:, :])
                pb = sm.tile([G, 16], f32, tag="pb")
                nc.vector.tensor_scalar_mul(out=pb[:, :], in0=pr[:, 16 * b:16 * (b + 1)],
                                            scalar1=ic_t[:, :])
                pbT_p = pm.tile([16, G], f32, tag="pbT", name=f"pbT_{b}", space="PSUM")
                nc.tensor.transpose(out=pbT_p[:, :], in_=pb[:, :],
                                    identity=ct["identf"][0:G, 0:G])
                pbT = sm.tile([16, G], f32, tag="pbTs")
                nc.vector.tensor_copy(out=pbT[:, :], in_=pbT_p[:, :])
                po_p = pm.tile([G, 8], f32, tag="po", name=f"po_{b}", space="PSUM")
                nc.tensor.matmul(out=po_p[:, :], lhsT=pbT[:, :], rhs=ct["W3"][:, :],
                                 start=True, stop=True)
                nc.vector.tensor_add(out=pooled_cat[:, 8 * b:8 * (b + 1)],
                                     in0=po_p[:, :], in1=ct["b3r"][:, :])
            pcT_p = pm.tile([16, G], f32, tag="pcT", space="PSUM")
            nc.tensor.transpose(out=pcT_p[:, :], in_=pooled_cat[:, :],
                                identity=ct["identf"][0:G, 0:G])
            pcT = sm.tile([16, G], f32, tag="pcTs")
            nc.vector.tensor_copy(out=pcT[:, :], in_=pcT_p[:, :])
            m1_p = pm.tile([G, 8], f32, tag="m1", space="PSUM")
            nc.tensor.matmul(out=m1_p[:, :], lhsT=pcT[:, :], rhs=ct["mW1"][:, :],
                             start=True, stop=True)
            m1_s = sm.tile([G, 8], f32, tag="m1s")
            nc.vector.tensor_add(out=m1_s[:, :], in0=m1_p[:, :], in1=ct["mb1r"][:, :])
            nc.vector.tensor_scalar_max(out=m1_s[:, :], in0=m1_s[:, :], scalar1=0.0)
            m1T_p = pm.tile([8, G], f32, tag="m1T", space="PSUM")
            nc.tensor.transpose(out=m1T_p[:, :], in_=m1_s[:, :],
                                identity=ct["identf"][0:G, 0:G])
            m1T = sm.tile([8, G], f32, tag="m1Ts")
            nc.vector.tensor_copy(out=m1T[:, :], in_=m1T_p[:, :])
            m2_p = pm.tile([G, 2], f32, tag="m2", space="PSUM")
            nc.tensor.matmul(out=m2_p[:, :], lhsT=m1T[:, :], rhs=ct["mW2"][:, :],
                             start=True, stop=True)
            m2_s = sm.tile([G, 2], f32, tag="m2s")
            nc.vector.tensor_add(out=m2_s[:, :], in0=m2_p[:, :], in1=ct["mb2r"][:, :])
            nc.sync.dma_start(out=out_p[:, :], in_=m2_s[:, :])
            pm_ctx.__exit__(None, None, None)

            latep_ctx.__exit__(None, None, None)
            midp_ctx.__exit__(None, None, None)

    nc.compile()
    return nc


# --------------------------------------------------------------------------
# driver
# --------------------------------------------------------------------------

def _run(inputs, trace=False):
    global last_results
    x = [np.asarray(inputs["x0"], np.float32), np.asarray(inputs["x1"], np.float32)]
    ei = [np.asarray(inputs["edge_index0"]), np.asarray(inputs["edge_index1"])]
    bt = [np.asarray(inputs["batch0"]), np.asarray(inputs["batch1"])]

    schs = [_prep_branch(ei[b], bt[b]) for b in range(2)]

    W1 = np.asarray(inputs["W1"], np.float32)
    b1 = np.asarray(inputs["b1"], np.float32)
    W2 = np.asarray(inputs["W2"], np.float32)
    b2 = np.asarray(inputs["b2"], np.float32)
    W1eo = np.concatenate([W1[:, 0::2], W1[:, 1::2]], axis=1).astype(bfloat16)
    common = dict(
        W1eo=W1eo,
        W1pi=W1[:, PI].astype(bfloat16),
        b1rep=np.broadcast_to(b1[PI], (P, 32)).astype(np.float32).copy(),
        W2p=np.asarray(W2[PI, :], np.float32).astype(bfloat16),
        b2rep=np.broadcast_to(b2, (P, 16)).astype(np.float32).copy(),
        W3=np.asarray(inputs["W3"], np.float32),
        b3r=np.broadcast_to(np.asarray(inputs["b3"], np.float32), (G, 8)).copy(),
        mW1=np.asarray(inputs["mW1"], np.float32),
        mb1r=np.broadcast_to(np.asarray(inputs["mb1"], np.float32), (G, 8)).copy(),
        mW2=np.asarray(inputs["mW2"], np.float32),
        mb2r=np.broadcast_to(np.asarray(inputs["mb2"], np.float32), (G, 2)).copy(),
        identf=np.eye(P, dtype=np.float32),
        iota=np.ascontiguousarray(
            np.broadcast_to(np.arange(P, dtype=np.float32), (P, P))).astype(bfloat16),
        ic0=schs[0]["inv_cnt"], ic1=schs[1]["inv_cnt"],
    )

    xpad = []
    diss = []
    for b in range(2):
        t = np.zeros((NPAD, 128), np.float32)
        t[:N] = x[b]
        xpad.append(t)
        diss.append(schs[b]["dis"])

    in_maps = []
    for c in range(NCORE):
        m = dict(common)
        for b in range(2):
            pc = schs[b]["per_core"][c]
            m[f"xT{b}"] = np.ascontiguousarray(
                (xpad[b][c * SH:(c + 1) * SH]
                 * diss[b][c * SH:(c + 1) * SH, None]).T).astype(bfloat16)
            m[f"idx{b}"] = pc["idx"]
            m[f"dl{b}"] = pc["dl"]
            m[f"dist{b}"] = pc["dist"]
            m[f"dist2{b}"] = pc["dist2"]
            m[f"DtT{b}"] = pc["DtT"]
        in_maps.append(m)

    nc = _build_program(schs)
    res = run_bass_kernel_spmd(nc, in_maps, list(range(NCORE)), trace=trace)
    last_results = res
    return np.asarray(res.results[0]["out"], np.float32)


def kernel(**inputs):
    return _run(inputs, trace=False)


# revision 16
# speedup vs baseline: 1.2146x; 1.0174x over previous
"""Trainium2 Bass kernel for the BGNN (3-layer GCN x 2 branches + mean-pool + MLP).

v2 design (ap_gather-based):
  - Nodes dst-sharded across 8 cores (SH=12544/core). Per branch, edges split
    into 8 gather streams by src%8; each GpSimd Q7 core gathers its stream's
    source features from an SBUF-resident replicated table via ap_gather
    (features transposed onto partitions, 8-node column packing).
  - L1 table bf16 feat-pairs in u32 units ([128, NT, 2] bf16); L2 table f32
    [128, NT]. Tables device-built per band, AllGathered, reloaded packed.
  - Per 128-edge-slot block: TensorE strided-plane transposes flip [feat,edge]
    to [edge,feat]; dl-vs-iota one-hot matmuls scatter into per-window PSUM
    accumulators (bank-first start, bank-last stop).
  - Layer 3 + mean-pool folded into host-precomputed structural matrix
    Dt[s,g]: pool = ((Dt^T @ act2)/cnt) @ W3 + b3.  MLP replicated per core.
  - Block->window schedule baked into the SPMD program: per-(stream,window)
    run capacities common across cores (max over cores, padded to 32);
    window-straddling blocks use multiple masked one-hot fragments.
"""
import sys

sys.path.insert(0, "/opt/trn_rl_repo")

import numpy as np
import ml_dtypes

import concourse.bacc as bacc
import concourse.bass as bass
import concourse.mybir as mybir
import concourse.tile as tile
from concourse.bass_utils import run_bass_kernel_spmd

P = 128
NCORE = 8
G = 64
N = 100000
SH = 12544
NPAD = SH * NCORE
NW = SH // P                # 98
NT = NPAD // 8              # 12544
CH = SH // 8                # 1568
KI = 4096
PI = np.concatenate([np.arange(0, 32, 2), np.arange(1, 32, 2)])

bfloat16 = ml_dtypes.bfloat16
last_results = None


def _pad_to(x, m):
    return (x + m - 1) // m * m


# --------------------------------------------------------------------------
# host prep
# --------------------------------------------------------------------------

def _prep_branch(ei, batch):
    src = ei[0].astype(np.int64)
    dst = ei[1].astype(np.int64)
    deg = np.bincount(dst, minlength=N).astype(np.float32) + 1.0
    dis = np.ones(NPAD, np.float32)
    dis[:N] = deg ** -0.5

    bpad = np.zeros(NPAD, np.int64)
    bpad[:N] = batch.astype(np.int64)
    flat = np.bincount(src * G + bpad[dst], weights=dis[dst].astype(np.float64),
                      minlength=NPAD * G)
    Dt = flat.reshape(NPAD, G).astype(np.float32)
    Dt *= dis[:, None]
    Dt[np.arange(N), bpad[:N]] += dis[:N] ** 2
    cnt = np.bincount(batch.astype(np.int64), minlength=G).astype(np.float32)
    inv_cnt = (1.0 / np.maximum(cnt, 1.0)).reshape(G, 1)

    core = dst // SH
    loc_s = src % SH
    stream = loc_s % 8
    gidx = (src // SH) * CH + loc_s // 8
    win = (dst % SH) // P
    dlv = (dst % P).astype(np.float32)

    key = (core * 8 + stream) * NW + win
    counts = np.bincount(key, minlength=NCORE * 8 * NW).reshape(NCORE, 8, NW)
    caps = _pad_to(counts.max(axis=0), 4)
    caps[0] = np.maximum(caps[0], 4)
    L = int(_pad_to(caps.sum(axis=1).max(), KI // 8 if False else P))
    NB = L // P

    starts = np.zeros((8, NW), np.int64)
    for q in range(8):
        np.cumsum(caps[q][:-1], out=starts[q][1:])

    # common block schedule
    frag_win = [[] for _ in range(NB * 8)]
    for q in range(8):
        for w in range(NW):
            s0, s1 = starts[q][w], starts[q][w] + caps[q][w]
            for k in range(s0 // P, min((s1 + P - 1) // P, NB)):
                frag_win[k * 8 + q].append(w)
    dlcols = []
    sched = []
    for t in range(NB * 8):
        kk, q = t // 8, t % 8
        ent = []
        for w in frag_win[t]:
            col = len(dlcols)
            s0, s1 = starts[q][w], starts[q][w] + caps[q][w]
            lo, hi = max(s0, kk * P), min(s1, (kk + 1) * P)
            dlcols.append((t, q, w, lo, hi))
            ent.append((col, w))
        sched.append(ent)
    TB = _pad_to(len(dlcols), 32)

    order = np.lexsort((win, stream, core))
    gidx_o, dl_o, win_o = gidx[order], dlv[order], win[order]
    core_o, stream_o = core[order], stream[order]
    cbounds = np.searchsorted(core_o, np.arange(NCORE + 1))
    per_core = []
    for c in range(NCORE):
        lo, hi = cbounds[c], cbounds[c + 1]
        gq, gw = stream_o[lo:hi], win_o[lo:hi]
        gi, gd = gidx_o[lo:hi], dl_o[lo:hi]
        idx_arr = np.zeros((8, L), np.int64)
        dl_full = np.full((8, NB * P), -1.0, np.float32)
        qb = np.searchsorted(gq, np.arange(9))
        for q in range(8):
            ql, qh = qb[q], qb[q + 1]
            wq, iq, dq = gw[ql:qh], gi[ql:qh], gd[ql:qh]
            wcnt = np.bincount(wq, minlength=NW)
            wstart = np.zeros(NW, np.int64)
            np.cumsum(wcnt[:-1], out=wstart[1:])
            pos = starts[q][wq] + (np.arange(qh - ql) - wstart[wq])
            idx_arr[q, pos] = iq
            dl_full[q, pos] = dq
        idx_tile = np.zeros((P, L // 16), np.int16)
        for g in range(8):
            idx_tile[16 * g:16 * g + 16, :] = \
                idx_arr[g].astype(np.int16).reshape(L // 16, 16).T
        dl_tile = np.full((P, TB), -1.0, np.float32)
        for col, (t, q, w, flo, fhi) in enumerate(dlcols):
            kk = t // 8
            seg = dl_full[q, kk * P:(kk + 1) * P].copy()
            mask = np.zeros(P, bool)
            mask[flo - kk * P:fhi - kk * P] = True
            seg[~mask] = -1.0
            dl_tile[:, col] = seg
        dis_t = np.ascontiguousarray(dis[c * SH:(c + 1) * SH].reshape(NW, P).T)
        disP = np.ascontiguousarray(dis[c * SH:(c + 1) * SH].reshape(CH, 8).T)
        DtT = np.ascontiguousarray(
            Dt[c * SH:(c + 1) * SH].reshape(NW, P, G).transpose(1, 0, 2)
            .reshape(P, NW * G)).astype(bfloat16)
        per_core.append(dict(idx=idx_tile, dl=dl_tile.astype(bfloat16),
                             dist=dis_t, dist2=dis_t * dis_t, disP=disP,
                             DtT=DtT))
    return dict(sched=sched, TB=TB, NB=NB, L=L, inv_cnt=inv_cnt,
                per_core=per_core, dis=dis)


# --------------------------------------------------------------------------
# device program
# --------------------------------------------------------------------------

def _build_program(schs):
    nc = bacc.Bacc()
    f32 = mybir.dt.float32
    bf16 = mybir.dt.bfloat16
    i16 = mybir.dt.int16

    prm = {}
    for b in (0, 1):
        sch = schs[b]
        prm[f"xT{b}"] = nc.declare_dram_parameter(f"xT{b}", [P, SH], bf16, isOutput=False)
        prm[f"idx{b}"] = nc.declare_dram_parameter(f"idx{b}", [P, sch["L"] // 16], i16, isOutput=False)
        prm[f"dl{b}"] = nc.declare_dram_parameter(f"dl{b}", [P, sch["TB"]], bf16, isOutput=False)
        prm[f"dist{b}"] = nc.declare_dram_parameter(f"dist{b}", [P, NW], f32, isOutput=False)
        prm[f"dist2{b}"] = nc.declare_dram_parameter(f"dist2{b}", [P, NW], f32, isOutput=False)
        prm[f"DtT{b}"] = nc.declare_dram_parameter(f"DtT{b}", [P, NW * G], bf16, isOutput=False)
        prm[f"ic{b}"] = nc.declare_dram_parameter(f"ic{b}", [G, 1], f32, isOutput=False)
    for nm, shp, dt in (
        ("W1eo", [P, 32], bf16), ("W1pi", [P, 32], bf16), ("b1rep", [P, 32], f32),
        ("W2p", [32, 16], bf16), ("b2rep", [P, 16], f32),
        ("W3", [16, 8], f32), ("b3r", [G, 8], f32),
        ("mW1", [16, 8], f32), ("mb1r", [G, 8], f32),
        ("mW2", [8, 2], f32), ("mb2r", [G, 2], f32),
        ("identf", [P, P], f32), ("iota", [P, P], bf16),
    ):
        prm[nm] = nc.declare_dram_parameter(nm, shp, dt, isOutput=False)
    out_p = nc.declare_dram_parameter("out", [G, 2], f32, isOutput=True)

    t1loc = [nc.dram_tensor(f"t1loc{b}", [P, 2 * CH], bf16) for b in (0, 1)]
    t1full = [nc.dram_tensor(f"t1full{b}", [NCORE * P, 2 * CH], bf16) for b in (0, 1)]
    t2loc = [nc.dram_tensor(f"t2loc{b}", [P, CH], f32) for b in (0, 1)]
    t2full = [nc.dram_tensor(f"t2full{b}", [NCORE * P, CH], f32) for b in (0, 1)]
    pool_in = nc.dram_tensor("pool_in", [G, 32], f32)
    pool_out = nc.dram_tensor("pool_out", [G, 32], f32)

    with tile.TileContext(nc) as tc:
        with (
            tc.tile_pool(name="const", bufs=1) as cp,
            tc.tile_pool(name="tabs", bufs=1) as tbp,
            tc.tile_pool(name="stream", bufs=2) as sp,
            tc.tile_pool(name="small", bufs=3) as sm,
            tc.tile_pool(name="auxp", bufs=1, space="PSUM") as auxp,
        ):
            ct = {}
            for nm in ("W1eo", "W1pi", "b1rep", "W2p", "b2rep", "W3", "b3r",
                       "mW1", "mb1r", "mW2", "mb2r", "identf", "iota"):
                t = cp.tile(list(prm[nm].shape), prm[nm].dtype, tag=nm, name=f"c_{nm}")
                nc.sync.dma_start(out=t[(slice(None),) * 2], in_=prm[nm][:, :])
                ct[nm] = t
            identb = cp.tile([P, P], bf16)
            nc.vector.tensor_copy(out=identb[:, :], in_=ct["identf"][:, :])

            dl_t = []
            for b in range(2):
                dlt = cp.tile([P, schs[b]["TB"]], bf16, tag=f"dl{b}", name=f"dl_t{b}")
                nc.sync.dma_start(out=dlt[:, :], in_=prm[f"dl{b}"][:, :])
                dl_t.append(dlt)
            dist_t, dist2_t = [], []
            for b in range(2):
                d1 = cp.tile([P, NW], f32, tag=f"dist{b}", name=f"dist_t{b}")
                nc.sync.dma_start(out=d1[:, :], in_=prm[f"dist{b}"][:, :])
                dist_t.append(d1)
                d2 = cp.tile([P, NW], f32, tag=f"dist2{b}", name=f"dist2_t{b}")
                nc.sync.dma_start(out=d2[:, :], in_=prm[f"dist2{b}"][:, :])
                dist2_t.append(d2)

            # =========== phase A: L1 table builds (both branches) ===========
            midp_ctx = tc.tile_pool(name="midp", bufs=1)
            midp = midp_ctx.__enter__()
            latep_ctx = tc.tile_pool(name="latep", bufs=1)
            latep = latep_ctx.__enter__()
            act1 = [midp.tile([P, NW * 32], bf16, tag=f"act1{b}", name=f"act1_{b}")
                    for b in range(2)]
            hself1 = [midp.tile([P, NW * 32], bf16, tag=f"hs1{b}", name=f"hself1_{b}")
                      for b in range(2)]
            hself2 = [None, None]
            act2 = [None, None]

            xtp_ctx = tc.tile_pool(name="xtp", bufs=1)
            xtp = xtp_ctx.__enter__()
            pap_ctx = tc.tile_pool(name="pap", bufs=2, space="PSUM")
            pap = pap_ctx.__enter__()
            for b in range(2):
                xT = xtp.tile([P, CH, 8], bf16, tag="xT", name=f"xT_{b}")
                nc.sync.dma_start(out=xT[:, :, :], in_=prm[f"xT{b}"][:, :])
                hs1f = sm.tile([P, 32], f32, tag="hs1f")
                for w in range(NW):
                    hp = pap.tile([P, 32], f32, tag="hp1", space="PSUM")
                    nc.tensor.matmul(out=hp[:, :], lhsT=xT[:, 16 * w:16 * (w + 1), :],
                                     rhs=ct["W1pi"][:, :], start=True, stop=True)
                    nc.vector.tensor_scalar_mul(out=hs1f[:, :], in0=hp[:, :],
                                                scalar1=dist_t[b][:, w:w + 1])
                    nc.vector.tensor_add(
                        out=hself1[b][:, w * 32:(w + 1) * 32], in0=hs1f[:, :],
                        in1=ct["b1rep"][:, :])
                bnd = midp.tile([16, CH, 2], bf16, tag="bnd", name=f"bnd_{b}")
                NCH = 8
                cw = CH // NCH
                for g in range(8):
                    for u in range(2):
                        for chk in range(NCH):
                            c0 = chk * cw
                            bp = pap.tile([16, cw], f32, tag="bp1", space="PSUM")
                            nc.tensor.matmul(
                                out=bp[:, :],
                                lhsT=ct["W1eo"][:, 16 * u:16 * u + 16],
                                rhs=xT[:, c0:c0 + cw, g],
                                start=True, stop=True)
                            nc.vector.tensor_copy(
                                out=bnd[:, c0:c0 + cw, u], in_=bp[:, :])
                    nc.sync.dma_start(out=t1loc[b][16 * g:16 * g + 16, :],
                                      in_=bnd[:, :, :])
                nc.gpsimd.collective_compute(
                    "AllGather", mybir.AluOpType.bypass,
                    replica_groups=[list(range(NCORE))],
                    ins=[t1loc[b][:, :]], outs=[t1full[b][:, :]])

            # =========== gather/scatter machinery ===========
            def gather_layer(b, layer, tab, aggp, tpool, hself, act_out,
                             scale_out):
                sch = schs[b]
                L, NB, sched = sch["L"], sch["NB"], sch["sched"]
                wdiv, wmul = (16, 32) if layer == 1 else (32, 16)
                first_gen, last_win = {}, {}
                for t in range(NB * 8):
                    for (col, w) in sched[t]:
                        gen = w // wdiv
                        if gen not in first_gen:
                            first_gen[gen] = col
                        last_win[w] = col
                ncall = (L + KI - 1) // KI
                oh_state = {"c0": -99999, "tile": None}
                gen_tiles = {}

                def post_window(w):
                    gen = w // wdiv
                    off = (w % wdiv) * wmul
                    ag = gen_tiles[gen]
                    tmp = sm.tile([P, 32], f32, tag="post", name=f"post_{b}_{layer}_{w}")
                    nc.vector.tensor_scalar_mul(
                        out=tmp[:, 0:wmul], in0=ag[:, off:off + wmul],
                        scalar1=dist_t[b][:, w:w + 1])
                    nc.vector.tensor_add(out=tmp[:, 0:wmul], in0=tmp[:, 0:wmul],
                                         in1=hself[:, w * wmul:(w + 1) * wmul])
                    nc.vector.tensor_scalar_max(
                        out=tmp[:, 0:wmul], in0=tmp[:, 0:wmul], scalar1=0.0)
                    if scale_out:
                        nc.vector.tensor_scalar_mul(
                            out=act_out[:, w * wmul:(w + 1) * wmul],
                            in0=tmp[:, 0:wmul], scalar1=dist_t[b][:, w:w + 1])
                    else:
                        nc.vector.tensor_copy(
                            out=act_out[:, w * wmul:(w + 1) * wmul],
                            in_=tmp[:, 0:wmul])

                for ci in range(ncall):
                    ni = min(KI, L - ci * KI)
                    idc = sp.tile([P, KI // 16], i16, tag="idc", bufs=3,
                                  name=f"idc_{b}_{layer}_{ci}")
                    nc.sync.dma_start(
                        out=idc[:, 0:ni // 16],
                        in_=prm[f"idx{b}"][:, ci * (KI // 16):ci * (KI // 16) + ni // 16])
                    if layer == 1:
                        msgs = sp.tile([P, KI, 2], bf16, tag="msgs",
                                       name=f"msgs1_{b}_{ci}")
                        nc.gpsimd.ap_gather(
                            out_ap=msgs[:, 0:ni, :], in_ap=tab[:, :, :],
                            idxs_ap=idc[:, 0:ni // 16],
                            channels=P, num_elems=NT, d=2, num_idxs=ni)
                    else:
                        msgs = sp.tile([P, KI], f32, tag="msgs",
                                       name=f"msgs2_{b}_{ci}")
                        nc.gpsimd.ap_gather(
                            out_ap=msgs[:, 0:ni], in_ap=tab[:, :],
                            idxs_ap=idc[:, 0:ni // 16],
                            channels=P, num_elems=NT, d=1, num_idxs=ni)
                    for kk in range(ni // P):
                        tbase = (ci * (KI // P) + kk) * 8
                        if layer == 1:
                            tp = tpool.tile([P, 2, P], bf16, tag="tp", space="PSUM")
                            for u in range(2):
                                nc.tensor.transpose(
                                    out=tp[:, u, :],
                                    in_=msgs[:, kk * P:(kk + 1) * P, u],
                                    identity=identb[:, :])
                            rhsT = sm.tile([P, 2, P], bf16, tag="rhsT")
                            nc.vector.tensor_copy(out=rhsT[:, :, :], in_=tp[:, :, :])
                        else:
                            tp = tpool.tile([P, P], f32, tag="tp", space="PSUM")
                            nc.tensor.transpose(
                                out=tp[:, :], in_=msgs[:, kk * P:(kk + 1) * P],
                                identity=ct["identf"][:, :])
                            rhsT = sm.tile([P, 2, P], bf16, tag="rhsT")
                            nc.vector.tensor_copy(out=rhsT[:, 0, :], in_=tp[:, :])
                        done_wins = []
                        for q in range(8):
                            for (col, w) in sched[tbase + q]:
                                if not (oh_state["c0"] <= col < oh_state["c0"] + 32):
                                    c0 = (col // 32) * 32
                                    oh = sp.tile([P, 32, P], bf16, tag="oh",
                                                 name=f"oh_{b}_{layer}_{c0}")
                                    nc.vector.tensor_tensor(
                                        out=oh[:, :, :],
                                        in0=dl_t[b][:, c0:c0 + 32, None].to_broadcast([P, 32, P]),
                                        in1=ct["iota"][:, None, :].to_broadcast([P, 32, P]),
                                        op=mybir.AluOpType.is_equal)
                                    oh_state["c0"] = c0
                                    oh_state["tile"] = oh
                                ohc = oh_state["tile"]
                                gen = w // wdiv
                                if gen not in gen_tiles:
                                    gen_tiles[gen] = aggp.tile(
                                        [P, 512], f32, tag=f"agg{gen % 4}",
                                        name=f"agg_{b}_{layer}_{gen}", space="PSUM")
                                ag = gen_tiles[gen]
                                off = (w % wdiv) * wmul
                                st = (first_gen[gen] == col)
                                if layer == 1:
                                    nc.tensor.matmul(
                                        out=ag[:, off:off + 32],
                                        lhsT=ohc[:, col - oh_state["c0"], :],
                                        rhs=rhsT[:, :, 16 * q:16 * q + 16],
                                        start=st, stop=(last_win[w] == col),
                                        skip_group_check=True)
                                else:
                                    nc.tensor.matmul(
                                        out=ag[:, off:off + 16],
                                        lhsT=ohc[:, col - oh_state["c0"], :],
                                        rhs=rhsT[:, 0, 16 * q:16 * q + 16],
                                        start=st, stop=(last_win[w] == col),
                                        skip_group_check=True)
                                if last_win[w] == col:
                                    done_wins.append(w)
                        for w in done_wins:
                            post_window(w)

            def build_tab2(b, latep):
                act1T = xtp.tile([32, CH, 8], bf16, tag="xT", name=f"act1T_{b}")
                for w in range(NW):
                    ap_ = auxp.tile([32, P], bf16, tag="aux", space="PSUM")
                    nc.tensor.transpose(out=ap_[:, :],
                                        in_=act1[b][:, w * 32:(w + 1) * 32],
                                        identity=identb[:, :])
                    nc.vector.tensor_copy(out=act1T[:, 16 * w:16 * (w + 1), :],
                                          in_=ap_[:, :])
                hs2 = latep.tile([P, NW * 16], bf16, tag=f"hs2_{b}", name=f"hself2_{b}")
                hs2f = sm.tile([P, 16], f32, tag="hs2f")
                for w in range(NW):
                    hp = auxp.tile([P, 16], f32, tag="aux", space="PSUM")
                    nc.tensor.matmul(out=hp[:, :],
                                     lhsT=act1T[:, 16 * w:16 * (w + 1), :],
                                     rhs=ct["W2p"][:, :], start=True, stop=True)
                    nc.vector.tensor_scalar_mul(out=hs2f[:, :], in0=hp[:, :],
                                                scalar1=dist_t[b][:, w:w + 1])
                    nc.vector.tensor_add(
                        out=hs2[:, w * 16:(w + 1) * 16], in0=hs2f[:, :],
                        in1=ct["b2rep"][:, :])
                bnd2 = midp.tile([16, CH], f32, tag="bnd", name=f"bnd2_{b}")
                NCH = 8
                cw = CH // NCH
                for g in range(8):
                    for chk in range(NCH):
                        c0 = chk * cw
                        bp = auxp.tile([16, cw], f32, tag="aux", space="PSUM")
                        nc.tensor.matmul(
                            out=bp[:, :], lhsT=ct["W2p"][:, :],
                            rhs=act1T[:, c0:c0 + cw, g],
                            start=True, stop=True)
                        nc.vector.tensor_copy(
                            out=bnd2[:, c0:c0 + cw], in_=bp[:, :])
                    nc.sync.dma_start(out=t2loc[b][16 * g:16 * g + 16, :],
                                      in_=bnd2[:, :])
                nc.gpsimd.collective_compute(
                    "AllGather", mybir.AluOpType.bypass,
                    replica_groups=[list(range(NCORE))],
                    ins=[t2loc[b][:, :]], outs=[t2full[b][:, :]])
                return hs2

            pap_ctx.__exit__(None, None, None)

            # =========== phase B: L1 gathers + L2 table builds ===========
            l1p_ctx = tc.tile_pool(name="l1p", bufs=1, space="PSUM")
            l1p = l1p_ctx.__enter__()
            tp1_ctx = tc.tile_pool(name="tp1p", bufs=2, space="PSUM")
            tp1p = tp1_ctx.__enter__()
            for b in range(2):
                tab1 = tbp.tile([P, NT, 2], bf16, tag="tabfull", name=f"tab1_{b}")
                for c in range(NCORE):
                    nc.sync.dma_start(out=tab1[:, c * CH:(c + 1) * CH, :],
                                      in_=t1full[b][c * P:(c + 1) * P, :])
                gather_layer(b, 1, tab1, l1p, tp1p, hself1[b], act1[b],
                             scale_out=True)
                hself2[b] = build_tab2(b, latep)
            xtp_ctx.__exit__(None, None, None)
            tp1_ctx.__exit__(None, None, None)
            l1p_ctx.__exit__(None, None, None)

            # =========== phase C: L2 gathers + pool partials ===========
            dtp_ctx = tc.tile_pool(name="dtp", bufs=1)
            dtp = dtp_ctx.__enter__()
            l2p_ctx = tc.tile_pool(name="l2p", bufs=1, space="PSUM")
            l2p = l2p_ctx.__enter__()
            tp2_ctx = tc.tile_pool(name="tp2p", bufs=2, space="PSUM")
            tp2p = tp2_ctx.__enter__()
            for b in range(2):
                tab2 = tbp.tile([P, NT], f32, tag="tabfull", name=f"tab2_{b}")
                for c in range(NCORE):
                    nc.sync.dma_start(out=tab2[:, c * CH:(c + 1) * CH],
                                      in_=t2full[b][c * P:(c + 1) * P, :])
                a2 = latep.tile([P, NW * 16], bf16, tag=f"act2{b}", name=f"act2_{b}")
                act2[b] = a2
                gather_layer(b, 2, tab2, l2p, tp2p, hself2[b], a2,
                             scale_out=False)
                DtT = dtp.tile([P, NW * G], bf16, tag="DtT", name=f"DtT_{b}")
                nc.sync.dma_start(out=DtT[:, :], in_=prm[f"DtT{b}"][:, :])
                pp = auxp.tile([G, 16], f32, tag="aux", name=f"poolp_{b}",
                               space="PSUM")
                for w in range(NW):
                    nc.tensor.matmul(out=pp[:, :],
                                     lhsT=DtT[:, w * G:(w + 1) * G],
                                     rhs=a2[:, w * 16:(w + 1) * 16],
                                     start=(w == 0), stop=(w == NW - 1))
                ps = sm.tile([G, 16], f32, tag="pools")
                nc.vector.tensor_copy(out=ps[:, :], in_=pp[:, :])
                nc.sync.dma_start(out=pool_in[:, 16 * b:16 * (b + 1)], in_=ps[:, :])
            tp2_ctx.__exit__(None, None, None)
            l2p_ctx.__exit__(None, None, None)
            dtp_ctx.__exit__(None, None, None)

            # =========== tail: AllReduce + pool scale + W3 + MLP ===========
            nc.gpsimd.collective_compute(
                "AllReduce", mybir.AluOpType.add,
                replica_groups=[list(range(NCORE))],
                ins=[pool_in[:, :]], outs=[pool_out[:, :]])
            pr = sm.tile([G, 32], f32, tag="pr")
            nc.sync.dma_start(out=pr[:, :], in_=pool_out[:, :])
            pm_ctx = tc.tile_pool(name="pm", bufs=1, space="PSUM")
            pm = pm_ctx.__enter__()
            pooled_cat = sm.tile([G, 16], f32, tag="pcat")
            for b in range(2):
                ic_t = sm.tile([G, 1], f32, tag="ic")
                nc.sync.dma_start(out=ic_t[:, :], in_=prm[f"ic{b}"][:, :])
                pb = sm.tile([G, 16], f32, tag="pb")
                nc.vector.tensor_scalar_mul(out=pb[:, :], in0=pr[:, 16 * b:16 * (b + 1)],
                                            scalar1=ic_t[:, :])
                pbT_p = pm.tile([16, G], f32, tag="pbT", name=f"pbT_{b}", space="PSUM")
                nc.tensor.transpose(out=pbT_p[:, :], in_=pb[:, :],
                                    identity=ct["identf"][0:G, 0:G])
                pbT = sm.tile([16, G], f32, tag="pbTs")
                nc.vector.tensor_copy(out=pbT[:, :], in_=pbT_p[:, :])
                po_p = pm.tile([G, 8], f32, tag="po", name=f"po_{b}", space="PSUM")
                nc.tensor.matmul(out=po_p[:, :], lhsT=pbT[:, :], rhs=ct["W3"][:, :],
                                 start=True, stop=True)
                nc.vector.tensor_add(out=pooled_cat[:, 8 * b:8 * (b + 1)],
                                     in0=po_p[:, :], in1=ct["b3r"][:, :])
            pcT_p = pm.tile([16, G], f32, tag="pcT", space="PSUM")
            nc.tensor.transpose(out=pcT_p[:, :], in_=pooled_cat[:, :],
                                identity=ct["identf"][0:G, 0:G])
            pcT = sm.tile([16, G], f32, tag="pcTs")
            nc.vector.tensor_copy(out=pcT[:, :], in_=pcT_p[:, :])
            m1_p = pm.tile([G, 8], f32, tag="m1", space="PSUM")
            nc.tensor.matmul(out=m1_p[:, :], lhsT=pcT[:, :], rhs=ct["mW1"][:, :],
                             start=True, stop=True)
            m1_s = sm.tile([G, 8], f32, tag="m1s")
            nc.vector.tensor_add(out=m1_s[:, :], in0=m1_p[:, :], in1=ct["mb1r"][:, :])
            nc.vector.tensor_scalar_max(out=m1_s[:, :], in0=m1_s[:, :], scalar1=0.0)
            m1T_p = pm.tile([8, G], f32, tag="m1T", space="PSUM")
            nc.tensor.transpose(out=m1T_p[:, :], in_=m1_s[:, :],
                                identity=ct["identf"][0:G, 0:G])
            m1T = sm.tile([8, G], f32, tag="m1Ts")
            nc.vector.tensor_copy(out=m1T[:, :], in_=m1T_p[:, :])
            m2_p = pm.tile([G, 2], f32, tag="m2", space="PSUM")
            nc.tensor.matmul(out=m2_p[:, :], lhsT=m1T[:, :], rhs=ct["mW2"][:, :],
                             start=True, stop=True)
            m2_s = sm.tile([G, 2], f32, tag="m2s")
            nc.vector.tensor_add(out=m2_s[:, :], in0=m2_p[:, :], in1=ct["mb2r"][:, :])
            nc.sync.dma_start(out=out_p[:, :], in_=m2_s[:, :])
            pm_ctx.__exit__(None, None, None)

            latep_ctx.__exit__(None, None, None)
            midp_ctx.__exit__(None, None, None)

    nc.compile()
    return nc


# --------------------------------------------------------------------------
# driver
# --------------------------------------------------------------------------

def _run(inputs, trace=False):
    global last_results
    x = [np.asarray(inputs["x0"], np.float32), np.asarray(inputs["x1"], np.float32)]
    ei = [np.asarray(inputs["edge_index0"]), np.asarray(inputs["edge_index1"])]
    bt = [np.asarray(inputs["batch0"]), np.asarray(inputs["batch1"])]

    schs = [_prep_branch(ei[b], bt[b]) for b in range(2)]

    W1 = np.asarray(inputs["W1"], np.float32)
    b1 = np.asarray(inputs["b1"], np.float32)
    W2 = np.asarray(inputs["W2"], np.float32)
    b2 = np.asarray(inputs["b2"], np.float32)
    W1eo = np.concatenate([W1[:, 0::2], W1[:, 1::2]], axis=1).astype(bfloat16)
    common = dict(
        W1eo=W1eo,
        W1pi=W1[:, PI].astype(bfloat16),
        b1rep=np.broadcast_to(b1[PI], (P, 32)).astype(np.float32).copy(),
        W2p=np.asarray(W2[PI, :], np.float32).astype(bfloat16),
        b2rep=np.broadcast_to(b2, (P, 16)).astype(np.float32).copy(),
        W3=np.asarray(inputs["W3"], np.float32),
        b3r=np.broadcast_to(np.asarray(inputs["b3"], np.float32), (G, 8)).copy(),
        mW1=np.asarray(inputs["mW1"], np.float32),
        mb1r=np.broadcast_to(np.asarray(inputs["mb1"], np.float32), (G, 8)).copy(),
        mW2=np.asarray(inputs["mW2"], np.float32),
        mb2r=np.broadcast_to(np.asarray(inputs["mb2"], np.float32), (G, 2)).copy(),
        identf=np.eye(P, dtype=np.float32),
        iota=np.ascontiguousarray(
            np.broadcast_to(np.arange(P, dtype=np.float32), (P, P))).astype(bfloat16),
        ic0=schs[0]["inv_cnt"], ic1=schs[1]["inv_cnt"],
    )

    xpad = []
    diss = []
    for b in range(2):
        t = np.zeros((NPAD, 128), np.float32)
        t[:N] = x[b]
        xpad.append(t)
        diss.append(schs[b]["dis"])

    in_maps = []
    for c in range(NCORE):
        m = dict(common)
        for b in range(2):
            pc = schs[b]["per_core"][c]
            m[f"xT{b}"] = np.ascontiguousarray(
                (xpad[b][c * SH:(c + 1) * SH]
                 * diss[b][c * SH:(c + 1) * SH, None]).T).astype(bfloat16)
            m[f"idx{b}"] = pc["idx"]
            m[f"dl{b}"] = pc["dl"]
            m[f"dist{b}"] = pc["dist"]
            m[f"dist2{b}"] = pc["dist2"]
            m[f"DtT{b}"] = pc["DtT"]
        in_maps.append(m)

    nc = _build_program(schs)
    res = run_bass_kernel_spmd(nc, in_maps, list(range(NCORE)), trace=trace)
    last_results = res
    return np.asarray(res.results[0]["out"], np.float32)


def kernel(**inputs):
    return _run(inputs, trace=False)
